# revision 26
# baseline (speedup 1.0000x reference)
"""Distributed causal attention head on 8 TRN2 NeuronCores.

Problem: B=4, S=4096, D_in=512, D_out=64 causal attention
  K/V/Q = X @ W; scores = Q@K^T (causal, /sqrt(64)); Z = softmax(scores)@V

Sharding: core c = 2*b + h handles batch b, seq-half h.
q-rows are interleaved at 128-row-block granularity (core h owns global
q-blocks {2j+h}), which makes the causal block schedule IDENTICAL on all
cores (SPMD-safe) and balances FLOPs exactly.  Every core loads the full
(transposed) K/V inputs of its batch and projects them locally.

The whole kernel is interleaved at q-chunk granularity so the PE never
idles >3.4us (HAM stays warm) and compute overlaps the input DMA stream:
for each chunk c: DMA xq[c], xk/xv[2c:2c+2] (separate small tiles ->
precise Tile deps), project Q/K/V for just those columns, PE-transpose
the new V blocks, then run the chunk's attention.  Matmul inputs bf16,
psum/softmax f32.  Scores are computed transposed ST[k,q] with KpT
parity-packed so score matmuls run as row-tiled K=64 PAIRS; exp on ACT
in groups of 3 kblocks (scale=1/8 folded, no max-subtraction:
|scores/8| < ~1.5); AV matmuls accumulate Z^T in PSUM with a
ones-column in Vp giving the softmax denominator for free; Z^T is
PE-transposed back to q-major and normalized with a per-partition
reciprocal + tensor_scalar_mul; output is q-major [2048, 64] f32.
"""

import numpy as np
import ml_dtypes

import concourse.bass as bass
import concourse.bacc as bacc
import concourse.mybir as mybir
import concourse.tile as tile

B, S, D, E = 4, 4096, 512, 64
PB = 128                      # partition block
NKB = S // PB                 # 32 k-blocks (global)
NLQ = NKB // 2                # 16 local q-blocks per core
NCH = 4                       # q-chunks of 512 per core
CHW = 512                     # q-chunk width
ND = D // PB                  # 4 d-slices
GRP = 2                       # kblocks per exp group
LAG = 4                       # ST->AV software pipeline depth (groups)
BF16 = mybir.dt.bfloat16
F32 = mybir.dt.float32
NPBF16 = ml_dtypes.bfloat16


def kparity(kb):
    """kblock -> (partition base, chunk idx, col) in parity-packed kpT."""
    return 64 * (kb % 2), kb // 4, PB * ((kb // 2) % 2)


def build_nc():
    nc = bacc.Bacc(None)

    xq_d = nc.declare_dram_parameter("xq", [D, S // 2], BF16, isOutput=False)
    xk_d = nc.declare_dram_parameter("xk", [D, S], BF16, isOutput=False)
    xv_d = nc.declare_dram_parameter("xv", [D, S], BF16, isOutput=False)
    wq_d = nc.declare_dram_parameter("wq", [D, E], BF16, isOutput=False)
    wk_d = nc.declare_dram_parameter("wk", [D, E], BF16, isOutput=False)
    wv_d = nc.declare_dram_parameter("wv", [D, E], BF16, isOutput=False)
    cm_d = nc.declare_dram_parameter("cmask", [8, PB, CHW], BF16, isOutput=False)
    id_d = nc.declare_dram_parameter("ident", [PB, PB], F32, isOutput=False)
    out_d = nc.declare_dram_parameter("out", [S // 2, E], F32, isOutput=True)

    with tile.TileContext(nc) as tc:
        with tc.tile_pool(name="persist", bufs=1) as pp, \
             tc.tile_pool(name="st_ps", bufs=3, space="PSUM") as stp, \
             tc.tile_pool(name="zt_ps", bufs=2, space="PSUM") as ztp, \
             tc.tile_pool(name="work", bufs=2 * LAG + 2) as wp, \
             tc.tile_pool(name="osb", bufs=3) as op:
            # ---- persistent SBUF tiles ----
            wq_sb = pp.tile([PB, ND * E], BF16, name="wq_sb", tag="wq_sb")
            wk_sb = pp.tile([PB, ND * E], BF16, name="wk_sb", tag="wk_sb")
            wv_sb = pp.tile([PB, ND * E], BF16, name="wv_sb", tag="wv_sb")
            mk_sb = pp.tile([PB, 8 * CHW], BF16, name="mk_sb", tag="mk_sb")
            idf_sb = pp.tile([PB, PB], F32, name="idf_sb", tag="idf_sb")
            idb_sb = pp.tile([PB, PB], BF16, name="idb_sb", tag="idb_sb")
            # per-half input tiles (one DMA each -> precise, cheap deps)
            xq_sb = [[pp.tile([PB, 2 * CHW], BF16, name=f"xq{d}_{g}", tag=f"xq{d}_{g}")
                      for g in range(2)] for d in range(ND)]
            xk_sb = [[pp.tile([PB, 4 * CHW], BF16, name=f"xk{d}_{g}", tag=f"xk{d}_{g}")
                      for g in range(2)] for d in range(ND)]
            xv_sb = [[pp.tile([PB, 4 * CHW], BF16, name=f"xv{d}_{g}", tag=f"xv{d}_{g}")
                      for g in range(2)] for d in range(ND)]
            # projected tensors, chunked
            qpT = [pp.tile([PB, CHW], BF16, name=f"qpT{c}", tag=f"qpT{c}")
                   for c in range(NCH)]                    # dup both halves
            kpT = [pp.tile([PB, 2 * PB], BF16, name=f"kpT{c}", tag=f"kpT{c}")
                   for c in range(2 * NCH)]                # parity-packed
            vpT = [pp.tile([E, CHW], BF16, name=f"vpT{c}", tag=f"vpT{c}")
                   for c in range(2 * NCH)]
            vp = [pp.tile([PB, E + 1], BF16, name=f"vp{s}", tag=f"vp{s}")
                  for s in range(NKB)]

            # ---- constant DMAs (one each, on the fast sync queue, first) ----
            for w_d, w_sb in ((wq_d, wq_sb), (wk_d, wk_sb), (wv_d, wv_sb)):
                nc.sync.dma_start(
                    out=w_sb[:].rearrange("p (d e) -> p d e", e=E),
                    in_=w_d.rearrange("(d p) e -> p d e", p=PB))
            for s in range(NKB):
                nc.vector.memset(vp[s][:], 1.0)   # ones column prefill

            def dma_inputs(g):
                """Issue input DMAs for half g: xq cols, xk/xv cols."""
                for d in range(ND):
                    nc.sync.dma_start(
                        out=xq_sb[d][g][:],
                        in_=xq_d[PB * d:PB * (d + 1), 2 * CHW * g:2 * CHW * (g + 1)])
                for d in range(ND):
                    nc.sync.dma_start(
                        out=xk_sb[d][g][:],
                        in_=xk_d[PB * d:PB * (d + 1), 4 * CHW * g:4 * CHW * (g + 1)])
                if g == 0:
                    nc.sync.dma_start(out=idf_sb[:], in_=id_d[:])
                    nc.vector.tensor_copy(idb_sb[:], idf_sb[:])
                    nc.sync.dma_start(
                        out=mk_sb[:].rearrange("p (m q) -> p m q", q=CHW),
                        in_=cm_d.rearrange("m p q -> p m q"))
                for d in range(ND):
                    nc.sync.dma_start(
                        out=xv_sb[d][g][:],
                        in_=xv_d[PB * d:PB * (d + 1), 4 * CHW * g:4 * CHW * (g + 1)])

            def vtrans(s):
                """PE-transpose one projected-V block to k-major + copy out."""
                vproj(s // 4)
                vt_ps = stp.tile([PB, E], BF16, tag="st")
                nc.tensor.transpose(vt_ps[:], vpT[s // 4][:, PB * (s % 4):PB * (s % 4 + 1)],
                                    idb_sb[0:E, 0:E])
                nc.vector.tensor_copy(vp[s][:, 0:E], vt_ps[:])

            def project(c):
                """Project Q chunk c and K/V chunks 2c, 2c+1 (V transposes
                are emitted later, interleaved between ST groups)."""
                g = c // 2
                qof = CHW * (c % 2)
                qp_ps = stp.tile([E, CHW], F32, tag="st")
                for d in range(ND):
                    nc.tensor.matmul(qp_ps[:], wq_sb[:, E * d:E * (d + 1)],
                                     xq_sb[d][g][:, qof:qof + CHW],
                                     start=(d == 0), stop=(d == ND - 1))
                nc.vector.tensor_copy(qpT[c][0:E, :], qp_ps[:])
                nc.scalar.copy(qpT[c][E:2 * E, :], qp_ps[:])
                for kc in (2 * c, 2 * c + 1):
                    kof = CHW * (kc % 4)
                    kp_ps = stp.tile([E, CHW], F32, tag="st")
                    for d in range(ND):
                        nc.tensor.matmul(kp_ps[:], wk_sb[:, E * d:E * (d + 1)],
                                         xk_sb[d][g][:, kof:kof + CHW],
                                         start=(d == 0), stop=(d == ND - 1))
                    for j in range(4):
                        kb = 4 * kc + j
                        pb, kch, col = kparity(kb)
                        assert kch == kc
                        nc.vector.tensor_copy(kpT[kc][pb:pb + E, col:col + PB],
                                              kp_ps[:, PB * j:PB * (j + 1)])
            vproj_done = set()

            def vproj(kc):
                """Lazily project V chunk kc (called at first vtrans use)."""
                if kc in vproj_done:
                    return
                vproj_done.add(kc)
                kof = CHW * (kc % 4)
                vq_ps = stp.tile([E, CHW], F32, tag="st")
                for d in range(ND):
                    nc.tensor.matmul(vq_ps[:], wv_sb[:, E * d:E * (d + 1)],
                                     xv_sb[d][kc // 4][:, kof:kof + CHW],
                                     start=(d == 0), stop=(d == ND - 1))
                nc.vector.tensor_copy(vpT[kc][:], vq_ps[:])

            def st_mm(st_ps, ji, kb, c):
                pb, kch, col = kparity(kb)
                nc.tensor.matmul(st_ps[:, CHW * ji:CHW * (ji + 1)],
                                 kpT[kch][pb:pb + E, col:col + PB],
                                 qpT[c][pb:pb + E, :],
                                 start=True, stop=True, tile_position=(pb, 0))

            # prologue: first half's DMA + first chunk's projections
            dma_inputs(0)
            project(0)

            norm_pend = None
            for c in range(NCH):
                nkb = 8 * c + 8
                zt_ps = ztp.tile([E + 1, CHW], F32, tag="zt")
                korder = list(range(0, nkb))
                groups = [korder[i:i + GRP] for i in range(0, nkb, GRP)]
                pend = []
                drain_state = {"n": 0}

                def drain_avs(p_et, p_kbs, nkb=nkb, zt_ps=zt_ps, c=c, ds=drain_state):
                    for kb in p_kbs:      # late vtrans, spread across groups
                        if kb >= 8 * c:
                            vtrans(kb)
                    for ji, kb in enumerate(p_kbs):
                        nc.tensor.matmul(
                            zt_ps[:], vp[kb][:],
                            p_et[:, CHW * ji:CHW * (ji + 1)],
                            start=(ds["n"] == 0),
                            stop=(ds["n"] == nkb - 1),
                            skip_group_check=True)
                        ds["n"] += 1

                if c == 0:
                    dma_inputs(1)   # stream second half's inputs early
                for gi, kbs in enumerate(groups):
                    gw = len(kbs) * CHW
                    st_ps = stp.tile([PB, GRP * CHW], F32, tag="st")
                    # pair of consecutive kblocks -> concurrent row-tiled MMs
                    if len(kbs) >= 2:
                        st_mm(st_ps, 0, kbs[0], c)
                        st_mm(st_ps, 1, kbs[1], c)
                        rest = range(2, len(kbs))
                    else:
                        rest = range(len(kbs))
                    for ji in rest:
                        st_mm(st_ps, ji, kbs[ji], c)
                    if len(pend) > LAG - 1:
                        drain_avs(*pend.pop(0))
                    et_sb = wp.tile([PB, GRP * CHW], BF16, tag="et")
                    nc.scalar.activation(
                        et_sb[:, :gw], st_ps[:, :gw],
                        mybir.ActivationFunctionType.Exp, scale=0.125)
                    for ji, kb in enumerate(kbs):
                        m = kb - 8 * c
                        if m >= 0:
                            nc.vector.tensor_mul(
                                et_sb[:, CHW * ji:CHW * (ji + 1)],
                                et_sb[:, CHW * ji:CHW * (ji + 1)],
                                mk_sb[:, CHW * m:CHW * (m + 1)])
                    pend.append((et_sb, kbs))
                for p in pend:
                    drain_avs(*p)
                zs_sb = wp.tile([E + 1, CHW], F32, tag="zs")
                nc.vector.tensor_copy(zs_sb[:], zt_ps[:])
                # project next chunk while exp/AV tail of this chunk drains
                if c + 1 < NCH:
                    project(c + 1)
                # normalize via transpose (denominator = col E)
                for j in range(4):
                    zn_ps = ztp.tile([PB, E + 1], F32, tag="zt")
                    nc.tensor.transpose(zn_ps[:], zs_sb[:, PB * j:PB * (j + 1)],
                                        idf_sb[0:E + 1, 0:E + 1])
                    rc_sb = wp.tile([PB, 1], F32, tag="rc")
                    nc.vector.reciprocal(rc_sb[:], zn_ps[:, E:E + 1])
                    o_sb = op.tile([PB, E], F32, tag="osb")
                    nc.vector.tensor_scalar_mul(o_sb[:], zn_ps[:, 0:E], rc_sb[:])
                    q0 = PB * (4 * c + j)
                    nc.gpsimd.dma_start(out=out_d[q0:q0 + PB, :], in_=o_sb[:])
    nc.finalize()
    return nc


def make_core_inputs(key_np, value_np, query_np, Wk, Wv, Wq):
    """Host-side sharding: returns in_maps list of 8 dicts."""
    bf = lambda a: np.ascontiguousarray(a).astype(NPBF16)
    in_maps = []
    for c in range(8):
        b, h = c // 2, c % 2
        qrows = np.concatenate(
            [np.arange(PB * (2 * j + h), PB * (2 * j + h) + PB) for j in range(NLQ)])
        # causal masks: mask m applies to kblock kb = 8c+m of every chunk;
        # section jj (q sub-block) has global q-block g = 8c+2jj+h,
        # class = m - 2jj - h: <0 keep, ==0 triangular, >0 zero.
        cmask = np.zeros((8, PB, CHW), dtype=np.float32)
        ki = np.arange(PB)[:, None]
        qi = np.arange(PB)[None, :]
        tri = (ki <= qi).astype(np.float32)
        for m in range(8):
            for jj in range(4):
                cls = m - 2 * jj - h
                blk = np.ones((PB, PB), np.float32) if cls < 0 else (
                    tri if cls == 0 else np.zeros((PB, PB), np.float32))
                cmask[m][:, PB * jj:PB * (jj + 1)] = blk
        in_maps.append({
            "xq": bf(query_np[b][qrows].T),
            "xk": bf(key_np[b].T),
            "xv": bf(value_np[b].T),
            "wq": bf(Wq), "wk": bf(Wk), "wv": bf(Wv),
            "cmask": bf(cmask),
            "ident": np.eye(PB, dtype=np.float32),
        })
    return in_maps


def assemble_output(results):
    """results: list of 8 dicts with 'out' [2048, 64] f32 -> Z [B,S,E]."""
    Z = np.zeros((B, S, E), dtype=np.float32)
    for c in range(8):
        b, h = c // 2, c % 2
        o = results[c]["out"]  # [2048, E] q-major
        for j in range(NLQ):
            g = 2 * j + h
            Z[b, PB * g:PB * (g + 1), :] = o[PB * j:PB * (j + 1), :]
    return Z


def kernel(key_inputs, value_inputs, query_inputs, Wk, Wv, Wq):
    from concourse.bass_utils import run_bass_kernel_spmd
    nc = build_nc()
    in_maps = make_core_inputs(np.asarray(key_inputs), np.asarray(value_inputs),
                               np.asarray(query_inputs), np.asarray(Wk),
                               np.asarray(Wv), np.asarray(Wq))
    res = run_bass_kernel_spmd(nc, in_maps, core_ids=list(range(8)))
    return assemble_output(res.results)


# revision 27
# speedup vs baseline: 1.1531x; 1.1531x over previous
"""Distributed causal attention head on 8 TRN2 NeuronCores.

Problem: B=4, S=4096, D_in=512, D_out=64 causal attention
  K/V/Q = X @ W; scores = Q@K^T (causal, /sqrt(64)); Z = softmax(scores)@V

Sharding: core c = 2*b + h handles batch b, seq-half h.
q-rows are interleaved at 128-row-block granularity (core h owns global
q-blocks {2j+h}), which makes the causal block schedule IDENTICAL on all
cores (SPMD-safe) and balances FLOPs exactly.  Every core loads the full
(transposed) K/V inputs of its batch and projects them locally.

The whole kernel is interleaved at q-chunk granularity so the PE never
idles >3.4us (HAM stays warm) and compute overlaps the input DMA stream:
for each chunk c: DMA xq[c], xk/xv[2c:2c+2] (separate small tiles ->
precise Tile deps), project Q/K/V for just those columns, PE-transpose
the new V blocks, then run the chunk's attention.  Matmul inputs bf16,
psum/softmax f32.  Scores are computed transposed ST[k,q] with KpT
parity-packed so score matmuls run as row-tiled K=64 PAIRS; exp on ACT
in groups of 3 kblocks (scale=1/8 folded, no max-subtraction:
|scores/8| < ~1.5); AV matmuls accumulate Z^T in PSUM with a
ones-column in Vp giving the softmax denominator for free; Z^T is
PE-transposed back to q-major and normalized with a per-partition
reciprocal + tensor_scalar_mul; output is q-major [2048, 64] f32.
"""

import numpy as np
import ml_dtypes

import concourse.bass as bass
import concourse.bacc as bacc
import concourse.mybir as mybir
import concourse.tile as tile

B, S, D, E = 4, 4096, 512, 64
PB = 128                      # partition block
NKB = S // PB                 # 32 k-blocks (global)
NLQ = NKB // 2                # 16 local q-blocks per core
NCH = 4                       # q-chunks of 512 per core
CHW = 512                     # q-chunk width
ND = D // PB                  # 4 d-slices
GRP = 2                       # kblocks per exp group
LAG = 4                       # ST->AV software pipeline depth (groups)
BF16 = mybir.dt.bfloat16
F32 = mybir.dt.float32
NPBF16 = ml_dtypes.bfloat16


def kparity(kb):
    """kblock -> (partition base, chunk idx, col) in parity-packed kpT."""
    return 64 * (kb % 2), kb // 4, PB * ((kb // 2) % 2)


def build_nc():
    nc = bacc.Bacc(None)

    xq_d = nc.declare_dram_parameter("xq", [D, S // 2], BF16, isOutput=False)
    xk_d = nc.declare_dram_parameter("xk", [D, S], BF16, isOutput=False)
    xv_d = nc.declare_dram_parameter("xv", [D, S], BF16, isOutput=False)
    wq_d = nc.declare_dram_parameter("wq", [D, E], BF16, isOutput=False)
    wk_d = nc.declare_dram_parameter("wk", [D, E], BF16, isOutput=False)
    wv_d = nc.declare_dram_parameter("wv", [D, E], BF16, isOutput=False)
    cm_d = nc.declare_dram_parameter("cmask", [8, PB, CHW], BF16, isOutput=False)
    id_d = nc.declare_dram_parameter("ident", [PB, PB], F32, isOutput=False)
    out_d = nc.declare_dram_parameter("out", [S // 2, E], F32, isOutput=True)

    with tile.TileContext(nc) as tc:
        with tc.tile_pool(name="persist", bufs=1) as pp, \
             tc.tile_pool(name="st_ps", bufs=2, space="PSUM") as stp, \
             tc.tile_pool(name="pj_ps", bufs=2, space="PSUM") as pjp, \
             tc.tile_pool(name="zt_ps", bufs=2, space="PSUM") as ztp, \
             tc.tile_pool(name="work", bufs=2 * LAG + 2) as wp, \
             tc.tile_pool(name="osb", bufs=3) as op:
            # ---- persistent SBUF tiles ----
            wq_sb = pp.tile([PB, ND * E], BF16, name="wq_sb", tag="wq_sb")
            wk_sb = pp.tile([PB, ND * E], BF16, name="wk_sb", tag="wk_sb")
            wv_sb = pp.tile([PB, ND * E], BF16, name="wv_sb", tag="wv_sb")
            mk_sb = pp.tile([PB, 8 * CHW], BF16, name="mk_sb", tag="mk_sb")
            idf_sb = pp.tile([PB, PB], F32, name="idf_sb", tag="idf_sb")
            idb_sb = pp.tile([PB, PB], BF16, name="idb_sb", tag="idb_sb")
            # per-half input tiles (one DMA each -> precise, cheap deps)
            xq_sb = [[pp.tile([PB, 2 * CHW], BF16, name=f"xq{d}_{g}", tag=f"xq{d}_{g}")
                      for g in range(2)] for d in range(ND)]
            xk_sb = [[pp.tile([PB, 4 * CHW], BF16, name=f"xk{d}_{g}", tag=f"xk{d}_{g}")
                      for g in range(2)] for d in range(ND)]
            xv_sb = [[pp.tile([PB, 4 * CHW], BF16, name=f"xv{d}_{g}", tag=f"xv{d}_{g}")
                      for g in range(2)] for d in range(ND)]
            # projected tensors, chunked
            qpT = [pp.tile([PB, CHW], BF16, name=f"qpT{c}", tag=f"qpT{c}")
                   for c in range(NCH)]                    # dup both halves
            kpT = [pp.tile([PB, 2 * PB], BF16, name=f"kpT{c}", tag=f"kpT{c}")
                   for c in range(2 * NCH)]                # parity-packed
            vpT = [pp.tile([E, CHW], BF16, name=f"vpT{c}", tag=f"vpT{c}")
                   for c in range(2 * NCH)]
            vp = [pp.tile([PB, E + 1], BF16, name=f"vp{s}", tag=f"vp{s}")
                  for s in range(NKB)]

            # ---- constant DMAs (one each, on the fast sync queue, first) ----
            for w_d, w_sb in ((wq_d, wq_sb), (wk_d, wk_sb), (wv_d, wv_sb)):
                nc.sync.dma_start(
                    out=w_sb[:].rearrange("p (d e) -> p d e", e=E),
                    in_=w_d.rearrange("(d p) e -> p d e", p=PB))
            for s in range(NKB):
                nc.vector.memset(vp[s][:], 1.0)   # ones column prefill

            def dma_inputs(g):
                """Issue input DMAs for half g: xq cols, xk/xv cols."""
                for d in range(ND):
                    nc.sync.dma_start(
                        out=xq_sb[d][g][:],
                        in_=xq_d[PB * d:PB * (d + 1), 2 * CHW * g:2 * CHW * (g + 1)])
                for d in range(ND):
                    nc.sync.dma_start(
                        out=xk_sb[d][g][:],
                        in_=xk_d[PB * d:PB * (d + 1), 4 * CHW * g:4 * CHW * (g + 1)])
                if g == 0:
                    nc.sync.dma_start(out=idf_sb[:], in_=id_d[:])
                    nc.vector.tensor_copy(idb_sb[:], idf_sb[:])
                    nc.sync.dma_start(
                        out=mk_sb[:].rearrange("p (m q) -> p m q", q=CHW),
                        in_=cm_d.rearrange("m p q -> p m q"))
                for d in range(ND):
                    nc.sync.dma_start(
                        out=xv_sb[d][g][:],
                        in_=xv_d[PB * d:PB * (d + 1), 4 * CHW * g:4 * CHW * (g + 1)])

            def vtrans(s):
                """PE-transpose one projected-V block to k-major + copy out."""
                vproj(s // 4)
                vt_ps = pjp.tile([PB, E], BF16, tag="pj")
                nc.tensor.transpose(vt_ps[:], vpT[s // 4][:, PB * (s % 4):PB * (s % 4 + 1)],
                                    idb_sb[0:E, 0:E])
                nc.vector.tensor_copy(vp[s][:, 0:E], vt_ps[:])

            def project(c):
                """Project Q chunk c and K/V chunks 2c, 2c+1 (V transposes
                are emitted later, interleaved between ST groups)."""
                g = c // 2
                qof = CHW * (c % 2)
                qp_ps = pjp.tile([E, CHW], F32, tag="pj")
                for d in range(ND):
                    nc.tensor.matmul(qp_ps[:], wq_sb[:, E * d:E * (d + 1)],
                                     xq_sb[d][g][:, qof:qof + CHW],
                                     start=(d == 0), stop=(d == ND - 1))
                nc.vector.tensor_copy(qpT[c][0:E, :], qp_ps[:])
                nc.scalar.copy(qpT[c][E:2 * E, :], qp_ps[:])
                for kc in (2 * c, 2 * c + 1):
                    kof = CHW * (kc % 4)
                    kp_ps = pjp.tile([E, CHW], F32, tag="pj")
                    for d in range(ND):
                        nc.tensor.matmul(kp_ps[:], wk_sb[:, E * d:E * (d + 1)],
                                         xk_sb[d][g][:, kof:kof + CHW],
                                         start=(d == 0), stop=(d == ND - 1))
                    for j in range(4):
                        kb = 4 * kc + j
                        pb, kch, col = kparity(kb)
                        assert kch == kc
                        nc.vector.tensor_copy(kpT[kc][pb:pb + E, col:col + PB],
                                              kp_ps[:, PB * j:PB * (j + 1)])
            vproj_done = set()

            def vproj(kc):
                """Lazily project V chunk kc (called at first vtrans use)."""
                if kc in vproj_done:
                    return
                vproj_done.add(kc)
                kof = CHW * (kc % 4)
                vq_ps = pjp.tile([E, CHW], F32, tag="pj")
                for d in range(ND):
                    nc.tensor.matmul(vq_ps[:], wv_sb[:, E * d:E * (d + 1)],
                                     xv_sb[d][kc // 4][:, kof:kof + CHW],
                                     start=(d == 0), stop=(d == ND - 1))
                nc.vector.tensor_copy(vpT[kc][:], vq_ps[:])

            def st_mm(st_ps, ji, kb, c):
                pb, kch, col = kparity(kb)
                nc.tensor.matmul(st_ps[:, CHW * ji:CHW * (ji + 1)],
                                 kpT[kch][pb:pb + E, col:col + PB],
                                 qpT[c][pb:pb + E, :],
                                 start=True, stop=True, tile_position=(pb, 0))

            # prologue: first half's DMA + first chunk's projections
            dma_inputs(0)
            project(0)

            norm_pend = None
            for c in range(NCH):
                nkb = 8 * c + 8
                zt_ps = ztp.tile([E + 1, CHW], F32, tag="zt")
                korder = list(range(0, nkb))
                groups = [korder[i:i + GRP] for i in range(0, nkb, GRP)]
                pend = []
                drain_state = {"n": 0}

                def drain_avs(p_et, p_kbs, nkb=nkb, zt_ps=zt_ps, c=c, ds=drain_state):
                    for kb in p_kbs:      # late vtrans, spread across groups
                        if kb >= 8 * c:
                            vtrans(kb)
                    for ji, kb in enumerate(p_kbs):
                        nc.tensor.matmul(
                            zt_ps[:], vp[kb][:],
                            p_et[:, CHW * ji:CHW * (ji + 1)],
                            start=(ds["n"] == 0),
                            stop=(ds["n"] == nkb - 1),
                            skip_group_check=True)
                        ds["n"] += 1

                if c == 0:
                    dma_inputs(1)   # stream second half's inputs early
                for gi, kbs in enumerate(groups):
                    gw = len(kbs) * CHW
                    st_ps = stp.tile([PB, GRP * CHW], F32, tag="st")
                    # pair of consecutive kblocks -> concurrent row-tiled MMs
                    if len(kbs) >= 2:
                        st_mm(st_ps, 0, kbs[0], c)
                        st_mm(st_ps, 1, kbs[1], c)
                        rest = range(2, len(kbs))
                    else:
                        rest = range(len(kbs))
                    for ji in rest:
                        st_mm(st_ps, ji, kbs[ji], c)
                    if len(pend) > LAG - 1:
                        drain_avs(*pend.pop(0))
                    et_sb = wp.tile([PB, GRP * CHW], BF16, tag="et")
                    nc.scalar.activation(
                        et_sb[:, :gw], st_ps[:, :gw],
                        mybir.ActivationFunctionType.Exp, scale=0.125)
                    for ji, kb in enumerate(kbs):
                        m = kb - 8 * c
                        if m >= 0:
                            nc.vector.tensor_mul(
                                et_sb[:, CHW * ji:CHW * (ji + 1)],
                                et_sb[:, CHW * ji:CHW * (ji + 1)],
                                mk_sb[:, CHW * m:CHW * (m + 1)])
                    pend.append((et_sb, kbs))
                for p in pend:
                    drain_avs(*p)
                zs_sb = wp.tile([E + 1, CHW], F32, tag="zs")
                nc.vector.tensor_copy(zs_sb[:], zt_ps[:])
                # project next chunk while exp/AV tail of this chunk drains
                if c + 1 < NCH:
                    project(c + 1)
                # normalize via transpose (denominator = col E)
                for j in range(4):
                    zn_ps = ztp.tile([PB, E + 1], F32, tag="zt")
                    nc.tensor.transpose(zn_ps[:], zs_sb[:, PB * j:PB * (j + 1)],
                                        idf_sb[0:E + 1, 0:E + 1])
                    rc_sb = wp.tile([PB, 1], F32, tag="rc")
                    nc.vector.reciprocal(rc_sb[:], zn_ps[:, E:E + 1])
                    o_sb = op.tile([PB, E], F32, tag="osb")
                    nc.vector.tensor_scalar_mul(o_sb[:], zn_ps[:, 0:E], rc_sb[:])
                    q0 = PB * (4 * c + j)
                    nc.gpsimd.dma_start(out=out_d[q0:q0 + PB, :], in_=o_sb[:])
    nc.finalize()
    return nc


def make_core_inputs(key_np, value_np, query_np, Wk, Wv, Wq):
    """Host-side sharding: returns in_maps list of 8 dicts."""
    bf = lambda a: np.ascontiguousarray(a).astype(NPBF16)
    in_maps = []
    for c in range(8):
        b, h = c // 2, c % 2
        qrows = np.concatenate(
            [np.arange(PB * (2 * j + h), PB * (2 * j + h) + PB) for j in range(NLQ)])
        # causal masks: mask m applies to kblock kb = 8c+m of every chunk;
        # section jj (q sub-block) has global q-block g = 8c+2jj+h,
        # class = m - 2jj - h: <0 keep, ==0 triangular, >0 zero.
        cmask = np.zeros((8, PB, CHW), dtype=np.float32)
        ki = np.arange(PB)[:, None]
        qi = np.arange(PB)[None, :]
        tri = (ki <= qi).astype(np.float32)
        for m in range(8):
            for jj in range(4):
                cls = m - 2 * jj - h
                blk = np.ones((PB, PB), np.float32) if cls < 0 else (
                    tri if cls == 0 else np.zeros((PB, PB), np.float32))
                cmask[m][:, PB * jj:PB * (jj + 1)] = blk
        in_maps.append({
            "xq": bf(query_np[b][qrows].T),
            "xk": bf(key_np[b].T),
            "xv": bf(value_np[b].T),
            "wq": bf(Wq), "wk": bf(Wk), "wv": bf(Wv),
            "cmask": bf(cmask),
            "ident": np.eye(PB, dtype=np.float32),
        })
    return in_maps


def assemble_output(results):
    """results: list of 8 dicts with 'out' [2048, 64] f32 -> Z [B,S,E]."""
    Z = np.zeros((B, S, E), dtype=np.float32)
    for c in range(8):
        b, h = c // 2, c % 2
        o = results[c]["out"]  # [2048, E] q-major
        for j in range(NLQ):
            g = 2 * j + h
            Z[b, PB * g:PB * (g + 1), :] = o[PB * j:PB * (j + 1), :]
    return Z


def kernel(key_inputs, value_inputs, query_inputs, Wk, Wv, Wq):
    from concourse.bass_utils import run_bass_kernel_spmd
    nc = build_nc()
    in_maps = make_core_inputs(np.asarray(key_inputs), np.asarray(value_inputs),
                               np.asarray(query_inputs), np.asarray(Wk),
                               np.asarray(Wv), np.asarray(Wq))
    res = run_bass_kernel_spmd(nc, in_maps, core_ids=list(range(8)))
    return assemble_output(res.results)


# revision 28
# speedup vs baseline: 1.1721x; 1.0165x over previous
"""Distributed causal attention head on 8 TRN2 NeuronCores.

Problem: B=4, S=4096, D_in=512, D_out=64 causal attention
  K/V/Q = X @ W; scores = Q@K^T (causal, /sqrt(64)); Z = softmax(scores)@V

Sharding: core c = 2*b + h handles batch b, seq-half h.
q-rows are interleaved at 128-row-block granularity (core h owns global
q-blocks {2j+h}), which makes the causal block schedule IDENTICAL on all
cores (SPMD-safe) and balances FLOPs exactly.  Every core loads the full
(transposed) K/V inputs of its batch and projects them locally.

The whole kernel is interleaved at q-chunk granularity so the PE never
idles >3.4us (HAM stays warm) and compute overlaps the input DMA stream:
for each chunk c: DMA xq[c], xk/xv[2c:2c+2] (separate small tiles ->
precise Tile deps), project Q/K/V for just those columns, PE-transpose
the new V blocks, then run the chunk's attention.  Matmul inputs bf16,
psum/softmax f32.  Scores are computed transposed ST[k,q] with KpT
parity-packed so score matmuls run as row-tiled K=64 PAIRS; exp on ACT
in groups of 3 kblocks (scale=1/8 folded, no max-subtraction:
|scores/8| < ~1.5); AV matmuls accumulate Z^T in PSUM with a
ones-column in Vp giving the softmax denominator for free; Z^T is
PE-transposed back to q-major and normalized with a per-partition
reciprocal + tensor_scalar_mul; output is q-major [2048, 64] f32.
"""

import numpy as np
import ml_dtypes

import concourse.bass as bass
import concourse.bacc as bacc
import concourse.mybir as mybir
import concourse.tile as tile

B, S, D, E = 4, 4096, 512, 64
PB = 128                      # partition block
NKB = S // PB                 # 32 k-blocks (global)
NLQ = NKB // 2                # 16 local q-blocks per core
NCH = 4                       # q-chunks of 512 per core
CHW = 512                     # q-chunk width
ND = D // PB                  # 4 d-slices
GRP = 2                       # kblocks per exp group
LAG = 4                       # ST->AV software pipeline depth (groups)
BF16 = mybir.dt.bfloat16
F32 = mybir.dt.float32
NPBF16 = ml_dtypes.bfloat16


def kparity(kb):
    """kblock -> (partition base, chunk idx, col) in parity-packed kpT."""
    return 64 * (kb % 2), kb // 4, PB * ((kb // 2) % 2)


def build_nc():
    nc = bacc.Bacc(None)

    xq_d = nc.declare_dram_parameter("xq", [D, S // 2], BF16, isOutput=False)
    xk_d = nc.declare_dram_parameter("xk", [D, S], BF16, isOutput=False)
    xv_d = nc.declare_dram_parameter("xv", [D, S], BF16, isOutput=False)
    wq_d = nc.declare_dram_parameter("wq", [D, E], BF16, isOutput=False)
    wk_d = nc.declare_dram_parameter("wk", [D, E], BF16, isOutput=False)
    wv_d = nc.declare_dram_parameter("wv", [D, E], BF16, isOutput=False)
    cm_d = nc.declare_dram_parameter("cmask", [8, PB, CHW], BF16, isOutput=False)
    id_d = nc.declare_dram_parameter("ident", [PB, PB], F32, isOutput=False)
    out_d = nc.declare_dram_parameter("out", [S // 2, E], F32, isOutput=True)

    with tile.TileContext(nc) as tc:
        with tc.tile_pool(name="persist", bufs=1) as pp, \
             tc.tile_pool(name="st_ps", bufs=2, space="PSUM") as stp, \
             tc.tile_pool(name="pj_ps", bufs=2, space="PSUM") as pjp, \
             tc.tile_pool(name="zt_ps", bufs=2, space="PSUM") as ztp, \
             tc.tile_pool(name="work", bufs=2 * LAG + 2) as wp, \
             tc.tile_pool(name="osb", bufs=3) as op:
            # ---- persistent SBUF tiles ----
            wq_sb = pp.tile([PB, ND * E], BF16, name="wq_sb", tag="wq_sb")
            wk_sb = pp.tile([PB, ND * E], BF16, name="wk_sb", tag="wk_sb")
            wv_sb = pp.tile([PB, ND * E], BF16, name="wv_sb", tag="wv_sb")
            mk_sb = pp.tile([PB, 8 * CHW], BF16, name="mk_sb", tag="mk_sb")
            idf_sb = pp.tile([PB, PB], F32, name="idf_sb", tag="idf_sb")
            idb_sb = pp.tile([PB, PB], BF16, name="idb_sb", tag="idb_sb")
            # per-half input tiles (one DMA each -> precise, cheap deps)
            xq_sb = [[pp.tile([PB, 2 * CHW], BF16, name=f"xq{d}_{g}", tag=f"xq{d}_{g}")
                      for g in range(2)] for d in range(ND)]
            xk_sb = [[pp.tile([PB, 4 * CHW], BF16, name=f"xk{d}_{g}", tag=f"xk{d}_{g}")
                      for g in range(2)] for d in range(ND)]
            xv_sb = [[pp.tile([PB, 4 * CHW], BF16, name=f"xv{d}_{g}", tag=f"xv{d}_{g}")
                      for g in range(2)] for d in range(ND)]
            # projected tensors, chunked
            qpT = [pp.tile([PB, CHW], BF16, name=f"qpT{c}", tag=f"qpT{c}")
                   for c in range(NCH)]                    # dup both halves
            kpT = [pp.tile([PB, 2 * PB], BF16, name=f"kpT{c}", tag=f"kpT{c}")
                   for c in range(2 * NCH)]                # parity-packed
            vpT = [pp.tile([E, CHW], BF16, name=f"vpT{c}", tag=f"vpT{c}")
                   for c in range(2 * NCH)]
            vp = [pp.tile([PB, E + 1], BF16, name=f"vp{s}", tag=f"vp{s}")
                  for s in range(NKB)]

            # ---- constant DMAs (one each, on the fast sync queue, first) ----
            for w_d, w_sb in ((wq_d, wq_sb), (wk_d, wk_sb), (wv_d, wv_sb)):
                nc.sync.dma_start(
                    out=w_sb[:].rearrange("p (d e) -> p d e", e=E),
                    in_=w_d.rearrange("(d p) e -> p d e", p=PB))
            for s in range(NKB):
                nc.vector.memset(vp[s][:], 1.0)   # ones column prefill

            def dma_inputs(g):
                """Issue input DMAs for half g: xq cols, xk/xv cols."""
                for d in range(ND):
                    nc.sync.dma_start(
                        out=xq_sb[d][g][:],
                        in_=xq_d[PB * d:PB * (d + 1), 2 * CHW * g:2 * CHW * (g + 1)])
                for d in range(ND):
                    nc.sync.dma_start(
                        out=xk_sb[d][g][:],
                        in_=xk_d[PB * d:PB * (d + 1), 4 * CHW * g:4 * CHW * (g + 1)])
                if g == 0:
                    nc.gpsimd.dma_start(out=idf_sb[:], in_=id_d[:])
                    nc.vector.tensor_copy(idb_sb[:], idf_sb[:])
                    nc.gpsimd.dma_start(
                        out=mk_sb[:].rearrange("p (m q) -> p m q", q=CHW),
                        in_=cm_d.rearrange("m p q -> p m q"))
                for d in range(ND):
                    nc.sync.dma_start(
                        out=xv_sb[d][g][:],
                        in_=xv_d[PB * d:PB * (d + 1), 4 * CHW * g:4 * CHW * (g + 1)])

            def vtrans(s):
                """PE-transpose one projected-V block to k-major + copy out."""
                vproj(s // 4)
                vt_ps = pjp.tile([PB, E], BF16, tag="pj")
                nc.tensor.transpose(vt_ps[:], vpT[s // 4][:, PB * (s % 4):PB * (s % 4 + 1)],
                                    idb_sb[0:E, 0:E])
                nc.vector.tensor_copy(vp[s][:, 0:E], vt_ps[:])

            def project(c):
                """Project Q chunk c and K/V chunks 2c, 2c+1 (V transposes
                are emitted later, interleaved between ST groups)."""
                g = c // 2
                qof = CHW * (c % 2)
                qp_ps = pjp.tile([E, CHW], F32, tag="pj")
                for d in range(ND):
                    nc.tensor.matmul(qp_ps[:], wq_sb[:, E * d:E * (d + 1)],
                                     xq_sb[d][g][:, qof:qof + CHW],
                                     start=(d == 0), stop=(d == ND - 1))
                nc.vector.tensor_copy(qpT[c][0:E, :], qp_ps[:])
                nc.scalar.copy(qpT[c][E:2 * E, :], qp_ps[:])
                for kc in (2 * c, 2 * c + 1):
                    kof = CHW * (kc % 4)
                    kp_ps = pjp.tile([E, CHW], F32, tag="pj")
                    for d in range(ND):
                        nc.tensor.matmul(kp_ps[:], wk_sb[:, E * d:E * (d + 1)],
                                         xk_sb[d][g][:, kof:kof + CHW],
                                         start=(d == 0), stop=(d == ND - 1))
                    for j in range(4):
                        kb = 4 * kc + j
                        pb, kch, col = kparity(kb)
                        assert kch == kc
                        nc.vector.tensor_copy(kpT[kc][pb:pb + E, col:col + PB],
                                              kp_ps[:, PB * j:PB * (j + 1)])
            vproj_done = set()

            def vproj(kc):
                """Lazily project V chunk kc (called at first vtrans use)."""
                if kc in vproj_done:
                    return
                vproj_done.add(kc)
                kof = CHW * (kc % 4)
                vq_ps = pjp.tile([E, CHW], F32, tag="pj")
                for d in range(ND):
                    nc.tensor.matmul(vq_ps[:], wv_sb[:, E * d:E * (d + 1)],
                                     xv_sb[d][kc // 4][:, kof:kof + CHW],
                                     start=(d == 0), stop=(d == ND - 1))
                nc.vector.tensor_copy(vpT[kc][:], vq_ps[:])

            def st_mm(st_ps, ji, kb, c):
                pb, kch, col = kparity(kb)
                nc.tensor.matmul(st_ps[:, CHW * ji:CHW * (ji + 1)],
                                 kpT[kch][pb:pb + E, col:col + PB],
                                 qpT[c][pb:pb + E, :],
                                 start=True, stop=True, tile_position=(pb, 0))

            # prologue: first half's DMA + first chunk's projections
            dma_inputs(0)
            project(0)

            norm_pend = None
            for c in range(NCH):
                nkb = 8 * c + 8
                zt_ps = ztp.tile([E + 1, CHW], F32, tag="zt")
                korder = list(range(0, nkb))
                groups = [korder[i:i + GRP] for i in range(0, nkb, GRP)]
                pend = []
                drain_state = {"n": 0}

                def drain_avs(p_et, p_kbs, nkb=nkb, zt_ps=zt_ps, c=c, ds=drain_state):
                    for kb in p_kbs:      # late vtrans, spread across groups
                        if kb >= 8 * c:
                            vtrans(kb)
                    for ji, kb in enumerate(p_kbs):
                        nc.tensor.matmul(
                            zt_ps[:], vp[kb][:],
                            p_et[:, CHW * ji:CHW * (ji + 1)],
                            start=(ds["n"] == 0),
                            stop=(ds["n"] == nkb - 1),
                            skip_group_check=True)
                        ds["n"] += 1

                if c == 0:
                    dma_inputs(1)   # stream second half's inputs early
                for gi, kbs in enumerate(groups):
                    gw = len(kbs) * CHW
                    st_ps = stp.tile([PB, GRP * CHW], F32, tag="st")
                    # pair of consecutive kblocks -> concurrent row-tiled MMs
                    if len(kbs) >= 2:
                        st_mm(st_ps, 0, kbs[0], c)
                        st_mm(st_ps, 1, kbs[1], c)
                        rest = range(2, len(kbs))
                    else:
                        rest = range(len(kbs))
                    for ji in rest:
                        st_mm(st_ps, ji, kbs[ji], c)
                    if len(pend) > LAG - 1:
                        drain_avs(*pend.pop(0))
                    et_sb = wp.tile([PB, GRP * CHW], BF16, tag="et")
                    nc.scalar.activation(
                        et_sb[:, :gw], st_ps[:, :gw],
                        mybir.ActivationFunctionType.Exp, scale=0.125)
                    for ji, kb in enumerate(kbs):
                        m = kb - 8 * c
                        if m >= 0:
                            nc.vector.tensor_mul(
                                et_sb[:, CHW * ji:CHW * (ji + 1)],
                                et_sb[:, CHW * ji:CHW * (ji + 1)],
                                mk_sb[:, CHW * m:CHW * (m + 1)])
                    pend.append((et_sb, kbs))
                for p in pend:
                    drain_avs(*p)
                zs_sb = wp.tile([E + 1, CHW], F32, tag="zs")
                nc.vector.tensor_copy(zs_sb[:], zt_ps[:])
                # project next chunk while exp/AV tail of this chunk drains
                if c + 1 < NCH:
                    project(c + 1)
                # normalize via transpose (denominator = col E)
                for j in range(4):
                    zn_ps = ztp.tile([PB, E + 1], F32, tag="zt")
                    nc.tensor.transpose(zn_ps[:], zs_sb[:, PB * j:PB * (j + 1)],
                                        idf_sb[0:E + 1, 0:E + 1])
                    rc_sb = wp.tile([PB, 1], F32, tag="rc")
                    nc.vector.reciprocal(rc_sb[:], zn_ps[:, E:E + 1])
                    o_sb = op.tile([PB, E], F32, tag="osb")
                    nc.vector.tensor_scalar_mul(o_sb[:], zn_ps[:, 0:E], rc_sb[:])
                    q0 = PB * (4 * c + j)
                    nc.gpsimd.dma_start(out=out_d[q0:q0 + PB, :], in_=o_sb[:])
    nc.finalize()
    return nc


def make_core_inputs(key_np, value_np, query_np, Wk, Wv, Wq):
    """Host-side sharding: returns in_maps list of 8 dicts."""
    bf = lambda a: np.ascontiguousarray(a).astype(NPBF16)
    in_maps = []
    for c in range(8):
        b, h = c // 2, c % 2
        qrows = np.concatenate(
            [np.arange(PB * (2 * j + h), PB * (2 * j + h) + PB) for j in range(NLQ)])
        # causal masks: mask m applies to kblock kb = 8c+m of every chunk;
        # section jj (q sub-block) has global q-block g = 8c+2jj+h,
        # class = m - 2jj - h: <0 keep, ==0 triangular, >0 zero.
        cmask = np.zeros((8, PB, CHW), dtype=np.float32)
        ki = np.arange(PB)[:, None]
        qi = np.arange(PB)[None, :]
        tri = (ki <= qi).astype(np.float32)
        for m in range(8):
            for jj in range(4):
                cls = m - 2 * jj - h
                blk = np.ones((PB, PB), np.float32) if cls < 0 else (
                    tri if cls == 0 else np.zeros((PB, PB), np.float32))
                cmask[m][:, PB * jj:PB * (jj + 1)] = blk
        in_maps.append({
            "xq": bf(query_np[b][qrows].T),
            "xk": bf(key_np[b].T),
            "xv": bf(value_np[b].T),
            "wq": bf(Wq), "wk": bf(Wk), "wv": bf(Wv),
            "cmask": bf(cmask),
            "ident": np.eye(PB, dtype=np.float32),
        })
    return in_maps


def assemble_output(results):
    """results: list of 8 dicts with 'out' [2048, 64] f32 -> Z [B,S,E]."""
    Z = np.zeros((B, S, E), dtype=np.float32)
    for c in range(8):
        b, h = c // 2, c % 2
        o = results[c]["out"]  # [2048, E] q-major
        for j in range(NLQ):
            g = 2 * j + h
            Z[b, PB * g:PB * (g + 1), :] = o[PB * j:PB * (j + 1), :]
    return Z


def kernel(key_inputs, value_inputs, query_inputs, Wk, Wv, Wq):
    from concourse.bass_utils import run_bass_kernel_spmd
    nc = build_nc()
    in_maps = make_core_inputs(np.asarray(key_inputs), np.asarray(value_inputs),
                               np.asarray(query_inputs), np.asarray(Wk),
                               np.asarray(Wv), np.asarray(Wq))
    res = run_bass_kernel_spmd(nc, in_maps, core_ids=list(range(8)))
    return assemble_output(res.results)
